# revision 1
# baseline (speedup 1.0000x reference)
"""Distillation-trainer loss kernel for Trainium2 (8 NeuronCores).

Computes  loss = mean((attn(q,k,v) - attn(q,ck,cv))**2)  for
q:[1,8,1024,128], k/v:[1,8,8192,128], ck/cv:[1,8,1024,128] fp32.

Sharding: one kv-head per core (h axis, 8 heads / 8 cores). Each core
computes its head's squared-error partial sums; the host adds the 8
partials and divides by the element count (the "all-reduce" of the
scalar loss).

Per-core algorithm (head h):
  - load K/Q/CK, transpose to [d, n] layout via PE (fp32 transpose,
    cast to bf16 on the PSUM->SBUF copy).
  - scoresT[n, q] = KT-tile.T @ QT on PE in bf16 (out fp32 PSUM).
  - exp on ACT:  expT = Exp(scoresT * 1/sqrt(d)) -> bf16 SBUF. No
    max-subtraction: scores ~ N(0,1); max over 8k samples < 5, exp
    stays < ~150 which is safely inside fp32/bf16 range.
  - PV: z'[q, 0:128] + S[q] in one accumulation: stationary = expT
    chunk [128n, 128q], moving = v' [128n, 129] where v' has a ones
    column appended; PSUM accumulates over the 64 n-tiles.
  - z = z'[:, :128] * (1 / z'[:, 128]) on DVE; same for compressed;
    (z - zc)^2 row-reduced into per-partition partials.
"""

import numpy as np

import concourse.bass as bass
import concourse.mybir as mybir
import concourse.tile as tile
from concourse import bacc
from concourse.masks import make_identity
from concourse.bass_utils import run_bass_kernel_spmd

F32 = mybir.dt.float32
BF16 = mybir.dt.bfloat16
FP8 = mybir.dt.float8e4     # e4m3: PV operands (exp probs, values)
AF = mybir.ActivationFunctionType
ALU = mybir.AluOpType

B, H, Q, N, NC, D = 1, 8, 1024, 8192, 1024, 128
N_CORES = 8
SCALE = 1.0 / float(np.sqrt(D))

QC = 256                   # q chunk width for the scores moving operand
N_QC = Q // QC             # 4
GT = 6                     # n-tiles per PSUM scores region / ACT call (3 banks)
NT = N // 128              # 64 teacher n-tiles
NCT = NC // 128            # 8 compressed n-tiles


def _emit(nc: bass.Bass, tc: tile.TileContext, qh, kh, vh, ckh, cvh, out_dram):
    ctxs = []

    def pool(**kw):
        p = tc.tile_pool(**kw)
        ctxs.append(p)
        return p.__enter__()

    pconst = pool(name="pconst", bufs=1)
    pstage = pool(name="pstage", bufs=4)
    pex = pool(name="pex", bufs=3)
    psmall = pool(name="psmall", bufs=4)
    psc = pool(name="psc", bufs=2, space="PSUM")
    pz = pool(name="pz", bufs=1, space="PSUM")

    # ---- persistent SBUF tensors ----
    ident = pconst.tile([128, 128], BF16, tag="ident")
    make_identity(nc, ident[:])

    kT = pconst.tile([128, NT, 128], BF16, tag="kT")       # [d, t, n]
    vb = pconst.tile([128, NT, 129], BF16, tag="vb")        # [n, t, d+1]
    qT = pconst.tile([128, Q], BF16, tag="qT")             # [d, q]
    ckT = pconst.tile([128, NCT, 128], BF16, tag="ckT")
    cvb = pconst.tile([128, NCT, 129], BF16, tag="cvb")
    zcomp = pconst.tile([128, Q // 128, 128], F32, tag="zcomp")  # [q, qt, d]
    accq = pconst.tile([128, Q // 128], F32, tag="accq")

    nc.gpsimd.memset(vb[:, :, 128:129], 1.0)
    nc.gpsimd.memset(cvb[:, :, 128:129], 1.0)

    # Warm the ACT exp table while prep DMAs run, so the ~2.7us
    # ACT_TABLE_LOAD is off the first real exp's critical path.
    warm = psmall.tile([128, 1], F32, tag="warm")
    nc.gpsimd.memset(warm[:], 0.0)
    warm2 = psmall.tile([128, 1], F32, tag="warm2")
    nc.scalar.activation(warm2[:], warm[:], AF.Exp)

    # ---- load + transpose K, load V (cast fp32 -> bf16) ----
    def load_transposed_chunk(src, dst, g, tag):
        # 512 rows of src -> dst[:, 4g:4g+4, :] in [d, t, n] layout:
        # DMA load, DVE cast to bf16, PE-transpose each 128x128 tile.
        # Transpose PSUM scratch borrows the scores pool's slots (tag
        # "sp") so prep+main stay within the 8 PSUM banks.
        stg = pstage.tile([128, 4, 128], F32, tag=tag)
        ap = src[g * 512:(g + 1) * 512, :].rearrange("(i p) d -> p i d", p=128)
        nc.sync.dma_start(out=stg[:], in_=ap)
        kb = pstage.tile([128, 4, 128], BF16, tag=tag + "b")
        nc.vector.tensor_copy(kb[:], stg[:])
        tp = psc.tile([128, 4, 128], BF16, tag="sp")
        for j in range(4):
            nc.tensor.transpose(tp[:, j, :], kb[:, j, :], ident[:])
        nc.vector.tensor_copy(dst[:, 4 * g:4 * g + 4, :], tp[:])

    def load_values_chunk(src, dst, g, tag):
        # 512 rows of src -> dst[:, 4g:4g+4, 0:128] bf16 ([n, t, d])
        stg = pstage.tile([128, 4, 128], F32, tag=tag)
        ap = src[g * 512:(g + 1) * 512, :].rearrange("(i p) d -> p i d", p=128)
        nc.sync.dma_start(out=stg[:], in_=ap)
        nc.vector.tensor_copy(dst[:, 4 * g:4 * g + 4, 0:128], stg[:])

    def load_transposed(src, dst, n_tiles, tag):
        for g in range(n_tiles // 4):
            load_transposed_chunk(src, dst, g, tag)

    def load_values(src, dst, n_tiles, tag):
        for g in range(n_tiles // 4):
            load_values_chunk(src, dst, g, tag)

    # q: [1024, 128] -> qT [128, 1024]
    stq = pstage.tile([128, 8, 128], F32, tag="stq")
    nc.sync.dma_start(out=stq[:], in_=qh[:, :].rearrange("(i p) d -> p i d", p=128))
    qb = pstage.tile([128, 8, 128], BF16, tag="stqb")
    nc.vector.tensor_copy(qb[:], stq[:])
    for gg in range(2):
        tp = psc.tile([128, 4, 128], BF16, tag="sp")
        for j in range(4):
            nc.tensor.transpose(tp[:, j, :], qb[:, 4 * gg + j, :], ident[:])
        nc.vector.tensor_copy(
            qT[:, 512 * gg:512 * (gg + 1)].rearrange("p (a b) -> p a b", a=4),
            tp[:])

    # Small compressed-side operands first: the compressed attention
    # phase needs only ck/cv/q (1.5 MB), and its compute hides the
    # 8 MB k/v stream, whose chunks are interleaved into the
    # compressed phase below.
    load_transposed(ckh, ckT, NCT, "stk")
    load_values(cvh, cvb, NCT, "stv")

    # ---- attention + softmax-PV for one q-chunk of 256 ----
    def attend(keysT, vals, n_tiles, qc):
        """Returns (za, zb) PSUM tiles [128, 129] = [z' | S] per q-half.
        Two separate tiles: PSUM accumulation-group tracking is bank-
        granular, so the two interleaved groups need distinct banks."""
        za = pz.tile([128, 129], F32, tag="za")
        zb = pz.tile([128, 129], F32, tag="zb")
        qs = qT[:, qc * QC:(qc + 1) * QC]

        def pv_ops(ex, t0, gn):
            ops = []
            for j in range(gn):
                t = t0 + j
                st = dict(start=(t == 0), stop=(t == n_tiles - 1))
                for c0, zp in ((0, za), (128, zb)):
                    ops.append(lambda j=j, c0=c0, zp=zp, st=st, t=t:
                               nc.tensor.matmul(zp[:], ex[:, j, c0:c0 + 128],
                                                vals[:, t, :], **st))
            return ops

        def emit_pv(ex, t0, gn):
            for op in pv_ops(ex, t0, gn):
                op()

        # Ramp group sizes: small first exp groups shrink the pipeline-
        # fill bubble (PE waits on the first ACT of each attend).
        sizes = []
        left = n_tiles
        for want in (2, 4):
            if left > GT:
                sizes.append(want)
                left -= want
        while left > 0:
            gn = min(GT, left)
            sizes.append(gn)
            left -= gn

        pending = None
        t0 = 0
        for gn in sizes:
            sp = psc.tile([128, GT, QC], F32, tag="sp")
            for j in range(gn):
                nc.tensor.matmul(sp[:, j, :], keysT[:, t0 + j, :], qs,
                                 start=True, stop=True)
            if pending is not None:
                emit_pv(*pending)
            ex = pex.tile([128, GT, QC], BF16, tag="ex")
            nc.scalar.activation(ex[:, 0:gn, :], sp[:, 0:gn, :], AF.Exp,
                                 scale=SCALE)
            pending = (ex, t0, gn)
            t0 += gn
        emit_pv(*pending)
        return za, zb

    # Phase 1: compressed attention for all q chunks; normalized zc
    # lands in SBUF (zcomp). The heavy k/v loads are interleaved per
    # qc so their DMA streams behind this phase's compute.
    for qc in range(N_QC):
        za, zb = attend(ckT, cvb, NCT, qc)
        for g in range(4 * qc, 4 * qc + 4):
            load_transposed_chunk(kh, kT, g, "stk")
        for h, zp in ((0, za), (1, zb)):
            qt = qc * 2 + h
            zr = psmall.tile([128, 129], F32, tag="zr")
            nc.vector.tensor_copy(zr[:], zp[:])
            inv = psmall.tile([128, 1], F32, tag="inv")
            nc.vector.reciprocal(inv[:], zr[:, 128:129])
            nc.vector.tensor_scalar_mul(zcomp[:, qt, :], zr[:, 0:128], inv[:])
        for g in range(4 * qc, 4 * qc + 4):
            load_values_chunk(vh, vb, g, "stv")

    # Phase 2: teacher attention + MSE partials against stored zc.
    for qc in range(N_QC):
        za, zb = attend(kT, vb, NT, qc)
        for h, zp in ((0, za), (1, zb)):
            qt = qc * 2 + h
            zr = psmall.tile([128, 129], F32, tag="zcr")
            nc.vector.tensor_copy(zr[:], zp[:])
            inv = psmall.tile([128, 1], F32, tag="inv")
            nc.vector.reciprocal(inv[:], zr[:, 128:129])
            zcn = psmall.tile([128, 128], F32, tag="zcn")
            nc.vector.tensor_scalar_mul(zcn[:], zr[:, 0:128], inv[:])
            d = psmall.tile([128, 128], F32, tag="d")
            nc.vector.tensor_sub(d[:], zcn[:], zcomp[:, qt, :])
            d2 = psmall.tile([128, 128], F32, tag="d2")
            nc.vector.tensor_mul(d2[:], d[:], d[:])
            nc.vector.reduce_sum(out=accq[:, qt:qt + 1], in_=d2[:],
                                 axis=mybir.AxisListType.X)

    nc.sync.dma_start(out=out_dram[:], in_=accq[:])

    for p in reversed(ctxs):
        p.__exit__(None, None, None)


_NC_CACHE = None


def build_nc():
    global _NC_CACHE
    if _NC_CACHE is not None:
        return _NC_CACHE
    nc = bacc.Bacc()
    qh = nc.declare_dram_parameter("queries", [Q, D], F32, isOutput=False)
    kh = nc.declare_dram_parameter("keys", [N, D], F32, isOutput=False)
    vh = nc.declare_dram_parameter("values", [N, D], F32, isOutput=False)
    ckh = nc.declare_dram_parameter("c_keys", [NC, D], F32, isOutput=False)
    cvh = nc.declare_dram_parameter("c_values", [NC, D], F32, isOutput=False)
    out = nc.declare_dram_parameter("loss_sums", [128, Q // 128], F32, isOutput=True)
    with tile.TileContext(nc) as tc:
        _emit(nc, tc, qh, kh, vh, ckh, cvh, out)
    nc.compile()
    _NC_CACHE = nc
    return nc


def make_in_maps(queries, keys, values, c_keys, c_values):
    in_maps = []
    for h in range(N_CORES):
        in_maps.append({
            "queries": np.ascontiguousarray(queries[0, h], dtype=np.float32),
            "keys": np.ascontiguousarray(keys[0, h], dtype=np.float32),
            "values": np.ascontiguousarray(values[0, h], dtype=np.float32),
            "c_keys": np.ascontiguousarray(c_keys[0, h], dtype=np.float32),
            "c_values": np.ascontiguousarray(c_values[0, h], dtype=np.float32),
        })
    return in_maps


def run_cores(in_maps, trace=False, **kw):
    nc = build_nc()
    return run_bass_kernel_spmd(nc, in_maps, list(range(N_CORES)),
                                trace=trace, **kw)


def kernel(queries, keys, values, c_keys, c_values):
    res = run_cores(make_in_maps(queries, keys, values, c_keys, c_values))
    total = sum(float(r["loss_sums"].astype(np.float64).sum())
                for r in res.results)
    loss = total / float(B * H * Q * D)
    return np.asarray(loss, dtype=np.float32)

